# revision 3
# baseline (speedup 1.0000x reference)
"""GQA attention (B=2, S=2048, DIM=2048, H=16, KVH=4, HD=128, RoPE, causal)
on 8 TRN2 NeuronCores.

Sharding: core c -> batch b = c//4, head-group g = c%4 (q heads 4g..4g+3,
which map exactly to kv head g). Each core computes the partial output
attn_heads @ wo_slice.T  ([S, DIM]); the host sums the 4 partials per batch.

Device layout (everything "transposed", feature-major; all DRAM tensors are
host-pre-arranged so every SBUF destination tile is ONE contiguous run per
partition -> 128 large DMA descriptors per transfer, minimal issue cost):
  xTp  [128, 16, S]    bf16  x[b].T as (t p) s -> p t s
  wqp  [128, 4, 16, HD] bf16 per-head even/odd-permuted, 1/sqrt(HD)-scaled
                             wq.T as (t p) (h j) -> p h t j  (head-major!)
  wkp  [128, 16, HD]   bf16  permuted wk.T  (t p) j -> p t j
  wvp  [128, 16, HD]   bf16  wv.T (not permuted; v is not roped)
  wop  [128, 4, DIM]   bf16  wo[:, cols].T as (t p) d -> p t d
  cosT [128, S]  bf16  [cos; cos] rope table, frequency-major, duplicated
  sinT [128, S]  bf16  [-sin; sin] sign-folded rope table

The per-head even/odd permutation (rows [0,2,..,126,1,3,..,127]) turns RoPE
pair-interleaving into contiguous half-partitions; q.k dot products are
invariant because q and k are permuted identically.

Attention is computed in transposed score layout: scoresT[k, q] so that
probsT feeds the PV matmul directly (lhsT = v natural layout) and attnT
falls out in [hd, q] = exactly the lhsT the output projection needs.

Schedule notes (v2):
  - DMA issue is split across BOTH HWDGE queues: Sync issues wk + the xT
    stream (dt0, dt1 singles then 7 pairs, depth-2 completion chain so the
    first tile gets full bandwidth); Scalar issues all other weights.
    Pre-arranged DRAM layouts make every transfer 128 descriptors.
  - HAM warm-up matmuls are gated on a GpSimd memset (GpSimd's sequencer
    wakes several us before DVE's), so the PE clock ramps during the DMA
    lead-in and K-proj starts hot.
  - Phase A ping-pongs two PSUM pools: K/Q1/Q3/V use pool-A (tag sc, 4
    bufs), Q0/Q2 use pool-B (at/at/op/op), so each projection's psums are
    freed by ropes exactly one step ahead of the next projection's needs
    and V-proj is never serialized behind Q3's ropes.
  - Attention q-chunks run SHORTEST-FIRST [0,512,1024,1536]: the shallow
    chunk-0 pipeline overlaps V-proj, each chunk's output projection
    (pure PE) overlaps the next chunk's ACT-bound score stream, and the
    final chunk's O-proj is a dense PE-only tail.
  - exp() is the per-tile throughput limit of the score stream (ACT,
    ~578ns vs 432ns of PE work per k-tile), so ACT is kept exp-only in
    phase B: O-proj PSUM->SBUF copies all go to DVE, output DMAs are all
    issued from Sync.
  - softmax denominators: DVE accumulates the (masked) probs tiles, then
    ONE ones-matmul per (head, chunk) gives the partition-replicated sum.
  - PSUM in phase B: scores rotate 4 bufs (tag sc), PV accumulators
    double-buffer (tag at), z + O-proj groups share tag op (2 bufs).
"""

import math
import sys

import numpy as np

try:
    import concourse.bacc as bacc  # noqa: F401
except ImportError:
    sys.path.insert(0, "/opt/trn_rl_repo")

import ml_dtypes
import concourse.bacc as bacc
import concourse.tile as tile
from concourse import mybir
from concourse.bass_utils import run_bass_kernel_spmd
from concourse.bass import _add_dep_helper

BF16 = mybir.dt.bfloat16
F32 = mybir.dt.float32

B, S, DIM = 2, 2048, 2048
H, KVH, HD = 16, 4, 128
N_CORES = 8
P = 128
D_T = DIM // P      # 16 contraction tiles
NH = H // KVH       # 4 q-heads per core
QC = 512            # q-chunk (matmul moving free dim)
QB = S // QC        # 4 q-chunks
S_T = S // P        # 16 s-tiles / k-tiles
N_WARM = 6          # dummy warm-up matmuls to ramp HAM

_cached = {}


def _build_nc():
    nc = bacc.Bacc("TRN2", target_bir_lowering=False, debug=False,
                   num_devices=N_CORES)
    xTp = nc.dram_tensor("xTp", [P, D_T, S], BF16, kind="ExternalInput").ap()
    wqp = nc.dram_tensor("wqp", [P, NH, D_T, HD], BF16,
                         kind="ExternalInput").ap()
    wkp = nc.dram_tensor("wkp", [P, D_T, HD], BF16, kind="ExternalInput").ap()
    wvp = nc.dram_tensor("wvp", [P, D_T, HD], BF16, kind="ExternalInput").ap()
    wop = nc.dram_tensor("wop", [P, NH, DIM], BF16, kind="ExternalInput").ap()
    cosT = nc.dram_tensor("cosT", [HD, S], BF16, kind="ExternalInput").ap()
    sinT = nc.dram_tensor("sinT", [HD, S], BF16, kind="ExternalInput").ap()
    out = nc.dram_tensor("out", [S, DIM], BF16, kind="ExternalOutput").ap()

    with tile.TileContext(nc) as tc:
        _build_kernel(tc, xTp, wqp, wkp, wvp, wop, cosT, sinT, out)
    nc.compile()
    return nc


def _build_kernel(tc, xTp, wqp, wkp, wvp, wop, cosT, sinT, out):
    nc = tc.nc
    Exp = mybir.ActivationFunctionType.Exp

    with (
        tc.tile_pool(name="const", bufs=1) as const,
        tc.tile_pool(name="big", bufs=1) as big,
        tc.tile_pool(name="rtmp", bufs=8) as rtmp,
        tc.tile_pool(name="probs", bufs=9) as probs_pool,
        tc.tile_pool(name="pracc", bufs=3) as pracc_pool,
        tc.tile_pool(name="attn", bufs=6) as attn_pool,
        tc.tile_pool(name="rz", bufs=3) as rz_pool,
        tc.tile_pool(name="osb", bufs=2) as osb_pool,
        tc.tile_pool(name="psA", bufs=4, space="PSUM") as psA,
        tc.tile_pool(name="psAt", bufs=2, space="PSUM") as psAt,
        tc.tile_pool(name="psOp", bufs=2, space="PSUM") as psOp,
    ):
        # ---- constants + HAM warm-up ----
        # memsets on GpSimd: its sequencer wakes earliest, so the dummy
        # matmuls (which only need `dum` initialized) start ramping the PE
        # clock governor several us before DVE would have allowed.
        ones = const.tile([P, P], BF16, name="ones")
        nc.gpsimd.memset(ones, 1.0)
        dum = const.tile([P, QC], BF16, name="dum")
        nc.gpsimd.memset(dum, 0.25)
        warm_ps = psA.tile([P, QC], F32, name="sc")
        for _ in range(N_WARM):
            nc.tensor.matmul(warm_ps[:, 0:384], lhsT=dum[:, 0:P],
                             rhs=dum[:, 0:384], start=True, stop=True)

        # ---- input DMAs ----
        # Sync queue: wk first (tiny, needed for the very first matmul),
        # then the xT stream as dt-singles (fast first arrival) followed by
        # pairs, with a depth-2 completion chain so ~2 transfers are in
        # flight and arrive in consumption order.
        wk_sb = big.tile([P, D_T, HD], BF16, name="wk")
        dma_wk = nc.sync.dma_start(out=wk_sb, in_=wkp)

        xt_sb = big.tile([P, D_T, S], BF16, name="xt")
        xt_tiles = {}
        for dt in range(D_T):
            for sc in range(QB):
                xt_tiles[(dt, sc)] = xt_sb[:, dt, sc * QC:(sc + 1) * QC]
        # transfer groups: dt [0], [1], [2,3], ..., [14,15]
        groups = [(0, 1), (1, 2)] + [(d, d + 2) for d in range(2, D_T, 2)]
        xp_dmas = []
        for gi, (d0, d1) in enumerate(groups):
            dma = nc.sync.dma_start(out=xt_sb[:, d0:d1, :],
                                    in_=xTp[:, d0:d1, :])
            if gi == 0:
                _add_dep_helper(dma.ins, dma_wk.ins, sync=True,
                                reason="x stream behind wk")
            if gi >= 2:
                _add_dep_helper(dma.ins, xp_dmas[gi - 2].ins, sync=True,
                                reason="stagger xT load")
            xp_dmas.append(dma)

        # Scalar (ACT) HWDGE queue: all remaining weights in parallel with
        # the Sync stream. ACT is otherwise idle this early.
        wq_sb = big.tile([P, NH, D_T, HD], BF16, name="wq")
        dma_wq0 = nc.scalar.dma_start(out=wq_sb[:, 0], in_=wqp[:, 0])
        cos_sb = const.tile([HD, S], BF16, name="cos")
        sin_sb = const.tile([HD, S], BF16, name="sin")
        d_cos = nc.scalar.dma_start(out=cos_sb, in_=cosT)
        _add_dep_helper(d_cos.ins, dma_wq0.ins, sync=True, reason="after wq0")
        d_sin = nc.scalar.dma_start(out=sin_sb, in_=sinT)
        dma_wq123 = nc.scalar.dma_start(out=wq_sb[:, 1:NH], in_=wqp[:, 1:NH])
        _add_dep_helper(dma_wq123.ins, d_cos.ins, sync=True,
                        reason="wq123 after tables")
        wv_sb = big.tile([P, D_T, HD], BF16, name="wv")
        dma_wv = nc.scalar.dma_start(out=wv_sb, in_=wvp)
        _add_dep_helper(dma_wv.ins, dma_wq123.ins, sync=True,
                        reason="wv after wq123")
        wo_sb = big.tile([P, NH, DIM], BF16, name="wo")
        dma_wo = nc.scalar.dma_start(out=wo_sb, in_=wop)
        _add_dep_helper(dma_wo.ins, dma_wv.ins, sync=True, reason="wo last")

        qT = big.tile([P, NH, S], BF16, name="qT")
        kT = big.tile([P, S], BF16, name="kT")
        v_sb = big.tile([P, S_T, HD], BF16, name="v")

        def rope(dst, ps, sc):
            """dst (bf16 [128,512] slice) <- rotate(ps).

            ACT stages ps to bf16 SBUF twice (straight + halves swapped via
            ScalarE partition-shifting copies); DVE then runs three
            full-width ops against the sign-folded tables:
            dst = st*[cos;cos] + sw*[-sin;sin]."""
            h = HD // 2
            st = rtmp.tile([P, QC], BF16, name="rst")
            sw = rtmp.tile([P, QC], BF16, name="rsw")
            nc.scalar.copy(out=st, in_=ps)
            nc.scalar.copy(out=sw[0:h, :], in_=ps[h:P, :])
            nc.scalar.copy(out=sw[h:P, :], in_=ps[0:h, :])
            cos_c = cos_sb[:, sc * QC:(sc + 1) * QC]
            sin_c = sin_sb[:, sc * QC:(sc + 1) * QC]
            t0 = rtmp.tile([P, QC], BF16, name="rt")
            t1 = rtmp.tile([P, QC], BF16, name="rt")
            nc.vector.tensor_mul(t0, st, cos_c)
            nc.vector.tensor_mul(t1, sw, sin_c)
            nc.vector.tensor_add(dst, t0, t1)

        # ---- K projection + Q head-0, dt-outer ----
        # K runs 4 dt-tiles ahead of Q-h0 so the PE starts as soon as the
        # first xT tile lands (wq0 arrives a bit later on the other queue).
        # K -> pool A (tag sc), Q0 -> pool B (at/at/op/op).
        kps = [psA.tile([P, QC], F32, name="sc") for _ in range(QB)]
        q0ps = [psAt.tile([P, QC], F32, name="at"),
                psAt.tile([P, QC], F32, name="at"),
                psOp.tile([P, QC], F32, name="op"),
                psOp.tile([P, QC], F32, name="op")]

        def kmm(dt):
            for sc in range(QB):
                nc.tensor.matmul(kps[sc], lhsT=wk_sb[:, dt, :],
                                 rhs=xt_tiles[(dt, sc)],
                                 start=(dt == 0), stop=(dt == D_T - 1))

        def q0mm(dt):
            for sc in range(QB):
                nc.tensor.matmul(q0ps[sc], lhsT=wq_sb[:, 0, dt, :],
                                 rhs=xt_tiles[(dt, sc)],
                                 start=(dt == 0), stop=(dt == D_T - 1))

        for dt in range(4):
            kmm(dt)
        for dt in range(4, D_T):
            kmm(dt)
            q0mm(dt - 4)
        for dt in range(D_T - 4, D_T):
            q0mm(dt)

        # K ropes first: K's psums (pool A, which Q1 needs) finished 4
        # dt-steps before q0's, so they drain while q0's last matmuls run.
        for sc in range(QB):
            rope(kT[:, sc * QC:(sc + 1) * QC], kps[sc], sc)
        for sc in range(QB):
            rope(qT[:, 0, sc * QC:(sc + 1) * QC], q0ps[sc], sc)

        # ---- Q heads 1..3, dt-outer per head, ping-ponging pools ----
        for hh in range(1, NH):
            if hh % 2 == 1:
                qps = [psA.tile([P, QC], F32, name="sc") for _ in range(QB)]
            else:
                qps = [psAt.tile([P, QC], F32, name="at"),
                       psAt.tile([P, QC], F32, name="at"),
                       psOp.tile([P, QC], F32, name="op"),
                       psOp.tile([P, QC], F32, name="op")]
            for dt in range(D_T):
                for sc in range(QB):
                    nc.tensor.matmul(
                        qps[sc], lhsT=wq_sb[:, hh, dt, :],
                        rhs=xt_tiles[(dt, sc)],
                        start=(dt == 0), stop=(dt == D_T - 1))
            for sc in range(QB):
                rope(qT[:, hh, sc * QC:(sc + 1) * QC], qps[sc], sc)

        # ---- V projection (natural [s, hd] layout) ----
        # Pool A rotation: slots are freed by Q3's ropes one step ahead;
        # pool B stays free for the chunk-0 attention stream that overlaps.
        for st in range(S_T):
            ps = psA.tile([P, QC], F32, name="sc")
            for dt in range(D_T):
                nc.tensor.matmul(
                    ps[:, 0:HD],
                    lhsT=xt_tiles[(dt, st // 4)][:, (st % 4) * P:(st % 4 + 1) * P],
                    rhs=wv_sb[:, dt, :],
                    start=(dt == 0), stop=(dt == D_T - 1))
            nc.scalar.copy(out=v_sb[:, st, :], in_=ps[:, 0:HD])

        # ---- attention + output projection, per q-chunk ----
        # Chunks run SHORTEST-first: the shallow chunk-0 pipeline overlaps
        # V-proj, each chunk's O-proj (pure PE) fills the PE idle time of
        # the next chunk's ACT-bound score stream, and the final (longest)
        # chunk leaves a dense PE-only O-proj tail.
        chunks = [(0, 512), (512, 512), (1024, 512), (1536, 512)]
        for ci, (q0, qw) in enumerate(chunks):
            nk = (q0 + qw) // P  # causal k-tiles for this q-chunk
            attn_tiles = []
            for hh in range(NH):
                at_ps = psAt.tile([P, qw], F32, name="at")
                pr_acc = pracc_pool.tile([P, qw], BF16, name="pracc")
                for k in range(nk):
                    # On diagonal tiles only columns q0+off.. are causally
                    # valid; narrow every stage to that width.
                    off = max(0, k * P - q0)
                    w = qw - off
                    diag = k * P >= q0
                    sc_ps = psA.tile([P, QC], F32, name="sc")
                    nc.tensor.matmul(sc_ps[:, 0:w], lhsT=kT[:, k * P:(k + 1) * P],
                                     rhs=qT[:, hh, q0 + off:q0 + qw],
                                     start=True, stop=True)
                    pr = probs_pool.tile([P, QC], BF16, name="pr")
                    nc.scalar.activation(out=pr[:, 0:w], in_=sc_ps[:, 0:w],
                                         func=Exp)
                    if diag:  # zero where c' < r
                        nc.gpsimd.affine_select(
                            out=pr[:, 0:w], in_=pr[:, 0:w],
                            compare_op=mybir.AluOpType.is_ge,
                            fill=0.0, base=0, pattern=[[1, w]],
                            channel_multiplier=-1)
                    nc.tensor.matmul(at_ps[:, off:qw], lhsT=v_sb[:, k, :],
                                     rhs=pr[:, 0:w],
                                     start=(k == 0), stop=(k == nk - 1))
                    if k == 0:
                        nc.vector.tensor_copy(out=pr_acc, in_=pr[:, 0:qw])
                    else:
                        nc.vector.tensor_add(pr_acc[:, off:qw],
                                             pr_acc[:, off:qw], pr[:, 0:w])
                z_ps = psOp.tile([P, qw], F32, name="op")
                nc.tensor.matmul(z_ps, lhsT=ones, rhs=pr_acc,
                                 start=True, stop=True)
                rz = rz_pool.tile([P, qw], F32, name="rz")
                nc.vector.reciprocal_approx_fast(out=rz, in_=z_ps)
                a_sb = attn_pool.tile([P, qw], BF16, name="attn")
                nc.vector.tensor_mul(a_sb, at_ps, rz)
                attn_tiles.append(a_sb)

            # Output projection for this chunk. All PSUM->SBUF copies on
            # DVE (ACT must stay exp-only while the next chunk's score
            # stream overlaps this). Output DMAs all issue from Sync.
            for st in range(qw // P):
                row0 = q0 + st * P
                o_sb = osb_pool.tile([P, DIM], BF16, name="osb")
                for dc in range(DIM // QC):
                    op_ps = psOp.tile([P, QC], F32, name="op")
                    for j in range(NH):
                        nc.tensor.matmul(
                            op_ps, lhsT=attn_tiles[j][:, st * P:(st + 1) * P],
                            rhs=wo_sb[:, j, dc * QC:(dc + 1) * QC],
                            start=(j == 0), stop=(j == NH - 1))
                    nc.vector.tensor_copy(out=o_sb[:, dc * QC:(dc + 1) * QC],
                                          in_=op_ps)
                    if dc == 1:
                        nc.sync.dma_start(out=out[row0:row0 + P, 0:2 * QC],
                                          in_=o_sb[:, 0:2 * QC])
                nc.sync.dma_start(out=out[row0:row0 + P, 2 * QC:DIM],
                                  in_=o_sb[:, 2 * QC:DIM])


def _get_nc():
    if "nc" not in _cached:
        _cached["nc"] = _build_nc()
    return _cached["nc"]


def _prep_in_maps(x, freqs_cis, wq, wk, wv, wo):
    bf = ml_dtypes.bfloat16
    perm = np.concatenate([np.arange(0, HD, 2), np.arange(1, HD, 2)])
    scale = 1.0 / math.sqrt(HD)
    wq_p = (wq.reshape(H, HD, DIM)[:, perm, :] * scale).astype(np.float32)
    wk_p = wk.reshape(KVH, HD, DIM)[:, perm, :]
    cos_h = np.ascontiguousarray(freqs_cis[:, :, 0].T)  # [64, S]
    sin_h = np.ascontiguousarray(freqs_cis[:, :, 1].T)
    cosT = np.concatenate([cos_h, cos_h], axis=0).astype(bf)   # [128, S]
    sinT = np.concatenate([-sin_h, sin_h], axis=0).astype(bf)

    def p_t_j(wT):  # [DIM, J] -> [P, D_T, J]  ((t p) j -> p t j)
        J = wT.shape[1]
        return np.ascontiguousarray(
            wT.reshape(D_T, P, J).transpose(1, 0, 2)).astype(bf)

    in_maps = []
    for c in range(N_CORES):
        b, g = c // KVH, c % KVH
        hq = slice(NH * g, NH * (g + 1))
        # [NH, HD, DIM] -> [DIM, NH*HD] -> per-head p t j, head-major
        wq_core = wq_p[hq].reshape(NH * HD, DIM).T  # [DIM, NH*HD]
        wqp = np.ascontiguousarray(
            wq_core.reshape(D_T, P, NH, HD).transpose(1, 2, 0, 3)).astype(bf)
        wo_core = wo[:, NH * HD * g:NH * HD * (g + 1)].T  # [NH*HD, DIM]
        wop = np.ascontiguousarray(
            wo_core.reshape(NH, HD, DIM).transpose(1, 0, 2)).astype(bf)
        in_maps.append({
            "xTp": p_t_j(np.ascontiguousarray(x[b].T)),
            "wqp": wqp,
            "wkp": p_t_j(np.ascontiguousarray(wk_p[g].T)),
            "wvp": p_t_j(np.ascontiguousarray(wv[g * HD:(g + 1) * HD].T)),
            "wop": wop,
            "cosT": cosT,
            "sinT": sinT,
        })
    return in_maps


def _reduce_outputs(results):
    out = np.zeros((B, S, DIM), np.float32)
    for c in range(N_CORES):
        out[c // KVH] += results[c]["out"].astype(np.float32)
    return out


def kernel(x, freqs_cis, wq, wk, wv, wo, _trace=False, _trace_kwargs=None):
    nc = _get_nc()
    x, freqs_cis, wq, wk, wv, wo = (
        np.asarray(a, np.float32) for a in (x, freqs_cis, wq, wk, wv, wo))
    in_maps = _prep_in_maps(x, freqs_cis, wq, wk, wv, wo)
    res = run_bass_kernel_spmd(nc, in_maps, core_ids=list(range(N_CORES)),
                               trace=_trace, **(_trace_kwargs or {}))
    out = _reduce_outputs(res.results)
    if _trace:
        _cached["last_exec_time_ns"] = res.exec_time_ns
        _cached["last_results"] = res
    return out
